# revision 25
# baseline (speedup 1.0000x reference)
"""Distributed Trainium2 Bass kernel for nn_AttentionD_12412455485977.

3D-windowed multi-head attention with relative-position bias:
  qkv = x @ w_qkv ; per-head attention with bias gathered from rel_table
  via the static relative-position index; out = attn_out @ w_out + b_out.

Sharding: head-parallel. Core c computes head c for both batches and the
partial out-projection attn_out_h @ w_out[h]; the host normalizes by the
softmax denominator (returned as column 128) and sums the 8 partial
[2*2048, 128] results (the natural unshard of a head-sharded contraction).

Key structure (v2):
- scores^T[j, i] tiles [128, 1024] = one z-plane j-pair (jt=2jj+g) x 512 i.
  Single K=32 matmul per j-tile (no quadrant replication needed).
- The bias is block-Toeplitz over z: slab class kk = 2*ic - jj + 7 (14
  classes). Three per-tile softmax routes balance the engines:
    I: PE injects the raw bias into PSUM via an fp8 DoubleRow identity
       matmul, then ACT computes exp(psum) directly.
    P: ACT computes exp(psum); GPSIMD multiplies by a precomputed
       exp(bias) slab.
    D: the DVE computes exp(s+b) via the EXP2N custom-op pair
       ((1 + (s+b)/8192)^8192 by repeated squaring), bias add fused.
- PV runs transposed: out_ps[i, dh] accumulates lhsT=expT (stationary)
  x rhs=vaug [128, 33] over j-tiles -- 33-row matmuls instead of
  512-row, cutting PE cost ~4x. The [i, 33] result is evicted to bf16,
  transposed back to [33, i] by the DMA xbar transpose engine, and
  projected on-device with waug [33, 129] (w_out + denominator
  passthrough). Unnormalized projections stream to HBM; the host does
  the denominator division, the cross-head sum, and + b_out.
"""

import os
import sys

import numpy as np

for _p in ("/opt/trn_rl_repo", "/root/.axon_site/_ro/trn_rl_repo"):
    if os.path.isdir(_p) and _p not in sys.path:
        sys.path.append(_p)

import ml_dtypes  # noqa: E402
import concourse.bass as bass  # noqa: E402
import concourse.tile as tile  # noqa: E402
from concourse import bacc, mybir  # noqa: E402
from concourse import dve_ops as _dvo  # noqa: E402
from concourse.dve_spec import (  # noqa: E402
    Spec, Src0, Src1, C0, One, lower, sq, _has_src1,
)
from concourse.dve_uop import DveOpSpec  # noqa: E402
from concourse.bass_utils import run_bass_kernel_spmd  # noqa: E402

SQ_A, SQ_B = 5, 8
NEXP = 1 << 13

BF16 = mybir.dt.bfloat16
F32 = mybir.dt.float32
FP8 = mybir.dt.float8e4
NPBF16 = ml_dtypes.bfloat16
NPFP8 = ml_dtypes.float8_e4m3fn

B = 2            # batches
N = 2048         # tokens per batch (= 8*16*16, z-major)
C = 128          # channels
HEADS = 8
DH = 32          # head dim
NCORES = 8

# softmax routes by bias slab class kk = 2*ic - jj + 7 in [0, 13]
D_SET = frozenset({0, 1, 2, 3, 12, 13})     # DVE custom exp (16 tiles)
P_SET = frozenset()                         # ACT exp + Pool mul
I_SET = frozenset(range(14)) - D_SET - P_SET  # PE inject + ACT exp (48)
D_SLOT = {kk: s for s, kk in enumerate(sorted(D_SET))}
P_SLOT = {kk: s for s, kk in enumerate(sorted(P_SET))}
I_SLOT = {kk: s for s, kk in enumerate(sorted(I_SET))}


def _register_op(name, spec, subdim=False):
    for op in _dvo.OPS:
        if op.name == name:
            return op
    shas = {}
    for ver in ("v3", "v4"):
        uops = lower(spec, ver=ver)
        shas[ver] = DveOpSpec(name=name, opcode=1, uops=uops,
                              rd1_en=_has_src1(spec)).sha(ver)
    op = _dvo.DveOp(name=name, spec=spec, subdim=subdim, uops_sha=shas)
    _dvo.OPS.append(op)
    _dvo.CUSTOM_DVE_SPECS[name] = spec
    _dvo._SUB_OPCODE_FOR_NAME[name] = (
        _dvo._CUSTOM_DVE_ROW_BASE + len(_dvo.OPS) - 1)
    return op


def _ref_exp2n_a(in0, in1, s0, s1, imm2):
    t = ((in0.astype(np.float32) + in1.astype(np.float32)) * np.float32(s0)
         + np.float32(1.0)).astype(np.float32)
    for _ in range(SQ_A):
        t = (t * t).astype(np.float32)
    return t


def _ref_exp2n_b(in0, in1, s0, s1, imm2):
    t = in0.astype(np.float32)
    for _ in range(SQ_B):
        t = (t * t).astype(np.float32)
    return t


def _build_ops():
    body_a = (Src0 + Src1) * C0 + One
    for _ in range(SQ_A):
        body_a = sq(body_a)
    op_a = _register_op("EXP2N_A_ANT", Spec(body=body_a,
                                            reference=_ref_exp2n_a))
    body_b = Src0
    for _ in range(SQ_B):
        body_b = sq(body_b)
    op_b = _register_op("EXP2N_B_ANT", Spec(body=body_b,
                                            reference=_ref_exp2n_b))
    return op_a, op_b


EXP2N_A, EXP2N_B = _build_ops()


def _route(ic, jj):
    kk = 2 * ic - jj + 7
    if kk in D_SET:
        return "D", kk
    if kk in P_SET:
        return "P", kk
    return "I", kk


def _jj_order(ic):
    """Spread D-route tiles between ACT-route tiles so the DVE and ACT
    softmax pipelines run concurrently."""
    djj = [jj for jj in range(8) if _route(ic, jj)[0] == "D"]
    ajj = [jj for jj in range(8) if _route(ic, jj)[0] != "D"]
    if not djj:
        return ajj
    nd, na = len(djj), len(ajj)
    pos_d = {int(t * (8 / nd) + (8 / nd) / 2) for t in range(nd)}
    out = []
    di = ai = 0
    for k in range(8):
        if k in pos_d and di < nd:
            out.append(djj[di]); di += 1
        elif ai < na:
            out.append(ajj[ai]); ai += 1
        else:
            out.append(djj[di]); di += 1
    return out


# ---------------------------------------------------------------------------
# device graph
# ---------------------------------------------------------------------------


def _build():
    nc = bacc.Bacc(None, target_bir_lowering=False, debug=False)

    xt_e = nc.declare_dram_parameter("xt", [C, B * N], BF16, isOutput=False)
    w3_e = nc.declare_dram_parameter("w3", [C, 96], BF16, isOutput=False)
    waug_e = nc.declare_dram_parameter("waug", [DH + 1, C + 1], BF16,
                                       isOutput=False)
    ident_e = nc.declare_dram_parameter("ident", [128, 256], FP8,
                                        isOutput=False)
    logb_e = nc.declare_dram_parameter("logb", [128, len(D_SLOT) * 1024],
                                       BF16, isOutput=False)
    expb_e = (nc.declare_dram_parameter("expb", [128, len(P_SLOT) * 1024],
                                        BF16, isOutput=False)
              if P_SLOT else None)
    binj_e = nc.declare_dram_parameter("binj", [128, len(I_SLOT) * 2048],
                                       FP8, isOutput=False)
    out_e = nc.declare_dram_parameter("out", [B * N, C + 1], F32,
                                      isOutput=True)

    with tile.TileContext(nc) as tc:
        with tc.tile_pool(name="persist", bufs=1) as persist:
            w3 = persist.tile([C, 96], BF16)
            nc.sync.dma_start(w3[:], w3_e[:])
            # PE p-state warm-up on dummy matmuls during the startup DMA wait
            with tc.tile_pool(name="warm", bufs=1, space="PSUM") as warm:
                wsb = persist.tile([128, 256], BF16)
                nc.vector.memset(wsb[:], 0.0)
                wps = warm.tile([128, 256], F32)
                for _ in range(20):
                    nc.tensor.matmul(wps[:], lhsT=wsb[:, 0:128], rhs=wsb[:],
                                     start=True, stop=True)
            xt = persist.tile([C, B * N], BF16)
            for ch in range(4):
                nc.sync.dma_start(xt[:, ch * 1024:(ch + 1) * 1024],
                                  xt_e[:, ch * 1024:(ch + 1) * 1024])
            waug = persist.tile([DH + 1, C + 1], BF16)
            nc.sync.dma_start(waug[:], waug_e[:])
            ident = persist.tile([128, 256], FP8)
            nc.sync.dma_start(ident[:], ident_e[:])

            # bias slabs, in first-use order (b0 walks kk 7..0, then 9,8,
            # 11,10, 13,12)
            logb = persist.tile([128, len(D_SLOT) * 1024], BF16)
            expb = (persist.tile([128, len(P_SLOT) * 1024], BF16)
                    if P_SLOT else None)
            binj = persist.tile([128, len(I_SLOT) * 2048], FP8)
            for kk in (7, 6, 5, 4, 3, 2, 1, 0, 9, 8, 11, 10, 13, 12):
                if kk in D_SLOT:
                    s = D_SLOT[kk]
                    nc.sync.dma_start(logb[:, s * 1024:(s + 1) * 1024],
                                      logb_e[:, s * 1024:(s + 1) * 1024])
                if kk in P_SLOT:
                    s = P_SLOT[kk]
                    nc.sync.dma_start(expb[:, s * 1024:(s + 1) * 1024],
                                      expb_e[:, s * 1024:(s + 1) * 1024])
                if kk in I_SLOT:
                    s = I_SLOT[kk]
                    nc.sync.dma_start(binj[:, s * 2048:(s + 1) * 2048],
                                      binj_e[:, s * 2048:(s + 1) * 2048])

            # preload the Exp activation table during phase 1
            scratch = persist.tile([128, 1], F32)
            nc.vector.memset(scratch[:], 0.0)
            nc.scalar.activation(scratch[:], scratch[:],
                                 mybir.ActivationFunctionType.Exp)

            qkT = [persist.tile([64, N], BF16, tag=f"qkT{b}", name=f"qkT{b}")
                   for b in range(B)]
            kT = [persist.tile([32, N], BF16, tag=f"kT{b}", name=f"kT{b}")
                  for b in range(B)]
            vaug = [persist.tile([128, 16 * 33], BF16, tag=f"vaug{b}",
                                 name=f"vaug{b}") for b in range(B)]

            # ---- phase 1: qkv projections -------------------------------
            with tc.tile_pool(name="ph1", bufs=2, space="PSUM") as ph1, \
                 tc.tile_pool(name="ph1v", bufs=2, space="PSUM") as ph1v:
                for b in range(B):
                    nc.gpsimd.memset(vaug[b][:], 1.0)
                    for hh in range(2):
                        qk_ps = ph1.tile([64, 1024], F32, tag="qk_ps",
                                         name="qk_ps")
                        for ch in range(2):
                            col = hh * 1024 + ch * 512
                            nc.tensor.matmul(
                                qk_ps[:, ch * 512:(ch + 1) * 512],
                                lhsT=w3[:, 0:64],
                                rhs=xt[:, b * N + col:b * N + col + 512],
                                start=True, stop=True)
                        # evict halves on ACT and DVE in parallel
                        if hh == 0:
                            nc.scalar.activation(
                                qkT[b][:, 0:1024], qk_ps[:],
                                mybir.ActivationFunctionType.Copy)
                        else:
                            nc.vector.tensor_copy(qkT[b][:, 1024:2048],
                                                  qk_ps[:])
                        # k to base partition 0 (PE needs lhsT/rhs co-based)
                        nc.gpsimd.dma_start(
                            kT[b][:, hh * 1024:(hh + 1) * 1024],
                            qkT[b][32:64, hh * 1024:(hh + 1) * 1024])
                    for tt in range(4):
                        v_ps = ph1v.tile([128, 128], F32, tag="v_ps",
                                         name="v_ps")
                        for u in range(4):
                            nt = tt * 4 + u
                            nc.tensor.matmul(
                                v_ps[:, u * 32:(u + 1) * 32],
                                lhsT=xt[:, b * N + nt * 128:
                                        b * N + (nt + 1) * 128],
                                rhs=w3[:, 64:96],
                                start=(u == 0), stop=(u == 3),
                                skip_group_check=True)
                        dst = vaug[b][:, tt * 132:(tt + 1) * 132]
                        dst = dst.rearrange("p (f c) -> p f c", f=4)[:, :, 0:DH]
                        src = v_ps[:].rearrange("p (f c) -> p f c", f=4)
                        nc.vector.tensor_copy(dst, src)

            # ---- phase 2: attention ------------------------------------
            with (
                tc.tile_pool(name="score", bufs=3, space="PSUM") as score_pool,
                tc.tile_pool(name="outps", bufs=1, space="PSUM") as out_pool,
                tc.tile_pool(name="proj", bufs=1, space="PSUM") as proj_pool,
                tc.tile_pool(name="sb2", bufs=3) as sb2,
                tc.tile_pool(name="sb3", bufs=2) as sb3,
            ):
                def epilogue_early(out_ps):
                    # evict the PV accumulator and launch the xbar transposes;
                    # returns the transposed tile for epilogue_late.
                    osb = sb3.tile([128, 512], BF16, tag="osb", name="osb")
                    ov = osb[:].rearrange("p (f c) -> p f c", f=4)[:, :, 0:DH + 1]
                    nc.vector.tensor_copy(
                        ov, out_ps[:].rearrange("p (f c) -> p f c", f=4))
                    tT = sb3.tile([128, 512], BF16, tag="tT", name="tT")
                    for ib in range(4):
                        nc.sync.dma_start_transpose(
                            tT[:, ib * 128:(ib + 1) * 128],
                            osb[:, ib * 128:(ib + 1) * 128])
                    return tT

                def epilogue_late(b, ic, tT):
                    for half in range(2):
                        proj_ps = proj_pool.tile([128, 2 * (C + 1)], F32,
                                                 tag="proj", name="proj")
                        for u in range(2):
                            it = 2 * half + u
                            nc.tensor.matmul(
                                proj_ps[:, u * (C + 1):(u + 1) * (C + 1)],
                                lhsT=tT[0:DH + 1, it * 128:(it + 1) * 128],
                                rhs=waug[:], start=True, stop=True)
                        psb = sb3.tile([128, 2 * (C + 1)], F32, tag="psb",
                                       name="psb")
                        nc.vector.tensor_copy(psb[:], proj_ps[:])
                        row = b * N + ic * 512 + 2 * half * 128
                        dst = out_e[row:row + 256, :].rearrange(
                            "(u p) c -> p u c", u=2)
                        nc.sync.dma_start(
                            dst, psb[:].rearrange("p (u c) -> p u c", u=2))

                steps = []
                for b in range(B):
                    for ic in range(4):
                        order = _jj_order(ic)
                        for s, jj in enumerate(order):
                            steps.append((b, ic, jj, s))
                out_ps_of = {}
                carries = []          # deferred consumer+PV closures (2-deep)
                pending_early = None  # (b, ic) chunk awaiting evict+transpose
                pending_late = None   # (b, ic, tT) awaiting proj+store
                for (b, ic, jj, s) in steps:
                    if s == 0:
                        out_ps_of[(b, ic)] = out_pool.tile(
                            [128, 4 * (DH + 1)], F32, name="out_ps",
                            tag="out_ps")
                    route, kk = _route(ic, jj)
                    score_ps = score_pool.tile([128, 1024], F32,
                                               name="score_ps", tag="score_ps")
                    for g in range(2):
                        jt = 2 * jj + g
                        nc.tensor.matmul(
                            score_ps[:, g * 512:(g + 1) * 512],
                            lhsT=kT[b][:, jt * 128:(jt + 1) * 128],
                            rhs=qkT[b][0:32, ic * 512:(ic + 1) * 512],
                            start=True, stop=(route != "I"),
                            skip_group_check=True)
                        if route == "I":
                            sl = I_SLOT[kk]
                            rhs = binj[:, sl * 2048 + g * 1024:
                                       sl * 2048 + (g + 1) * 1024]
                            nc.tensor.matmul(
                                score_ps[:, g * 512:(g + 1) * 512],
                                lhsT=ident[:].rearrange("p (e m) -> p e m",
                                                        e=2),
                                rhs=rhs.rearrange("p (e n) -> p e n", e=2),
                                start=False, stop=True,
                                perf_mode=mybir.MatmulPerfMode.DoubleRow,
                                skip_group_check=True)
                    if len(carries) >= 2:
                        carries.pop(0)()
                    if s == 0 and carries:
                        # drain the previous chunk's tail so its epilogue can
                        # launch with maximal transpose headroom
                        carries.pop(0)()
                    if pending_early is not None and s >= 1:
                        pb, pic = pending_early
                        tT = epilogue_early(out_ps_of[(pb, pic)])
                        pending_late = (pb, pic, tT)
                        pending_early = None
                    if pending_late is not None and s >= 6:
                        epilogue_late(*pending_late)
                        pending_late = None

                    def emit_rest(b=b, ic=ic, jj=jj, s=s, route=route, kk=kk,
                                  score_ps=score_ps):
                        expT = sb2.tile([128, 1024], BF16, tag="expT",
                                        name="expT")
                        if route == "I":
                            nc.scalar.activation(
                                expT[:], score_ps[:],
                                mybir.ActivationFunctionType.Exp)
                        elif route == "P":
                            expS = sb2.tile([128, 1024], BF16, tag="expS",
                                            name="expS", bufs=2)
                            nc.scalar.activation(
                                expS[:], score_ps[:],
                                mybir.ActivationFunctionType.Exp)
                            sl = P_SLOT[kk]
                            nc.gpsimd.tensor_mul(
                                expT[:], expS[:],
                                expb[:, sl * 1024:(sl + 1) * 1024])
                        else:
                            t32 = sb2.tile([128, 1024], F32, tag="t32",
                                           name="t32", bufs=2)
                            sl = D_SLOT[kk]
                            nc.vector._custom_dve(
                                EXP2N_A, out=t32[:], in0=score_ps[:],
                                in1=logb[:, sl * 1024:(sl + 1) * 1024],
                                s0=1.0 / NEXP)
                            nc.vector._custom_dve(EXP2N_B, out=expT[:],
                                                  in0=t32[:])
                        out_ps = out_ps_of[(b, ic)]
                        for g in range(2):
                            jt = 2 * jj + g
                            for ib in range(4):
                                nc.tensor.matmul(
                                    out_ps[:, ib * (DH + 1):
                                           (ib + 1) * (DH + 1)],
                                    lhsT=expT[:, g * 512 + ib * 128:
                                              g * 512 + (ib + 1) * 128],
                                    rhs=vaug[b][:, jt * 33:(jt + 1) * 33],
                                    start=(s == 0 and g == 0 and ib == 0),
                                    stop=(s == 7 and g == 1 and ib == 3),
                                    skip_group_check=True)

                    carries.append(emit_rest)
                    if s == 7:
                        pending_early = (b, ic)
                for c in carries:
                    c()
                pb, pic = pending_early
                tT = epilogue_early(out_ps_of[(pb, pic)])
                if pending_late is not None:
                    epilogue_late(*pending_late)
                epilogue_late(pb, pic, tT)

    nc.compile()
    return nc


_NC = None


def _get_nc():
    global _NC
    if _NC is None:
        _NC = _build()
    return _NC


# ---------------------------------------------------------------------------
# host side
# ---------------------------------------------------------------------------

D3, H3, W3 = 8, 16, 16


def _bias_tile(table_h, kk):
    """Raw bias values for slab class kk: [128 p, 2 g, 512 ih] -> [128, 1024].

    scoresT tile: partition p = j within j-tile, col g*512+ih; j-token =
    jj*256 + g*128 + p (z-plane jj), i-token = ic*512 + ih.
    dz + 7 = kk + (ih >= 256).
    """
    p = np.arange(128)[:, None, None]
    g = np.arange(2)[None, :, None]
    ih = np.arange(512)[None, None, :]
    dz7 = kk + (ih // 256)
    pj = g * 128 + p
    yj, xj = pj // 16, pj % 16
    pi = ih % 256
    yi, xi = pi // 16, pi % 16
    idx = dz7 * ((2 * H3 - 1) * (2 * W3 - 1)) + (yi - yj + 15) * (2 * W3 - 1) \
        + (xi - xj + 15)
    return table_h[idx].reshape(128, 1024)


def _prep_in_maps(x, w_qkv, rel_table, w_out, b_out):
    x = np.asarray(x, np.float32)
    w_qkv = np.asarray(w_qkv, np.float32)
    rel_table = np.asarray(rel_table, np.float32)
    w_out = np.asarray(w_out, np.float32)

    scale = DH ** -0.5
    xt = np.ascontiguousarray(
        x.transpose(2, 0, 1).reshape(C, B * N)).astype(NPBF16)

    ident = np.zeros((128, 2, 128), np.float32)
    ident[:, 0, :] = np.eye(128)
    ident = np.ascontiguousarray(ident.reshape(128, 256)).astype(NPFP8)

    in_maps = []
    for hc in range(NCORES):
        w3 = np.concatenate([
            w_qkv[:, hc * DH:(hc + 1) * DH] * scale,
            w_qkv[:, 256 + hc * DH: 256 + (hc + 1) * DH],
            w_qkv[:, 512 + hc * DH: 512 + (hc + 1) * DH],
        ], axis=1).astype(NPBF16)
        waug = np.zeros((DH + 1, C + 1), np.float32)
        waug[0:DH, 0:C] = w_out[hc * DH:(hc + 1) * DH, :]
        waug[DH, C] = 1.0
        th = rel_table[:, hc]
        logb = np.zeros((128, len(D_SLOT) * 1024), np.float32)
        for kk, s in D_SLOT.items():
            logb[:, s * 1024:(s + 1) * 1024] = _bias_tile(th, kk)
        binj = np.zeros((128, len(I_SLOT), 2, 2, 512), np.float32)
        for kk, s in I_SLOT.items():
            binj[:, s, :, 0, :] = _bias_tile(th, kk).reshape(128, 2, 512)
        binj = np.ascontiguousarray(
            binj.reshape(128, len(I_SLOT) * 2048)).astype(NPFP8)
        im = {
            "xt": xt,
            "w3": np.ascontiguousarray(w3),
            "waug": waug.astype(NPBF16),
            "ident": ident,
            "logb": logb.astype(NPBF16),
            "binj": binj,
        }
        if P_SLOT:
            expb = np.zeros((128, len(P_SLOT) * 1024), np.float32)
            for kk, s in P_SLOT.items():
                expb[:, s * 1024:(s + 1) * 1024] = np.exp(_bias_tile(th, kk))
            im["expb"] = expb.astype(NPBF16)
        in_maps.append(im)
    return in_maps


def _run(in_maps, **kwargs):
    nc = _get_nc()
    return run_bass_kernel_spmd(nc, in_maps, core_ids=list(range(NCORES)),
                                **kwargs)


def _postprocess(res, b_out):
    acc = np.zeros((B * N, C), np.float64)
    for i in range(NCORES):
        o = res.results[i]["out"].astype(np.float64)
        acc += o[:, 0:C] / o[:, C:C + 1]
    acc += np.asarray(b_out, np.float64)[None, :]
    return acc.reshape(B, N, C).astype(np.float32)


def kernel(x, w_qkv, rel_table, w_out, b_out, d=None, h=None, w=None):
    in_maps = _prep_in_maps(x, w_qkv, rel_table, w_out, b_out)
    res = _run(in_maps)
    return _postprocess(res, b_out)


# revision 27
# speedup vs baseline: 1.0278x; 1.0278x over previous
"""Distributed Trainium2 Bass kernel for nn_AttentionD_12412455485977.

3D-windowed multi-head attention with relative-position bias:
  qkv = x @ w_qkv ; per-head attention with bias gathered from rel_table
  via the static relative-position index; out = attn_out @ w_out + b_out.

Sharding: head-parallel. Core c computes head c for both batches and the
partial out-projection attn_out_h @ w_out[h]; the host normalizes by the
softmax denominator (returned as column 128) and sums the 8 partial
[2*2048, 128] results (the natural unshard of a head-sharded contraction).

Key structure (v2):
- scores^T[j, i] tiles [128, 1024] = one z-plane j-pair (jt=2jj+g) x 512 i.
  Single K=32 matmul per j-tile (no quadrant replication needed).
- The bias is block-Toeplitz over z: slab class kk = 2*ic - jj + 7 (14
  classes). Three per-tile softmax routes balance the engines:
    I: PE injects the raw bias into PSUM via an fp8 DoubleRow identity
       matmul, then ACT computes exp(psum) directly.
    P: ACT computes exp(psum); GPSIMD multiplies by a precomputed
       exp(bias) slab.
    D: the DVE computes exp(s+b) via the EXP2N custom-op pair
       ((1 + (s+b)/8192)^8192 by repeated squaring), bias add fused.
- PV runs transposed: out_ps[i, dh] accumulates lhsT=expT (stationary)
  x rhs=vaug [128, 33] over j-tiles -- 33-row matmuls instead of
  512-row, cutting PE cost ~4x. The [i, 33] result is evicted to bf16,
  transposed back to [33, i] by the DMA xbar transpose engine, and
  projected on-device with waug [33, 129] (w_out + denominator
  passthrough). Unnormalized projections stream to HBM; the host does
  the denominator division, the cross-head sum, and + b_out.
"""

import os
import sys

import numpy as np

for _p in ("/opt/trn_rl_repo", "/root/.axon_site/_ro/trn_rl_repo"):
    if os.path.isdir(_p) and _p not in sys.path:
        sys.path.append(_p)

import ml_dtypes  # noqa: E402
import concourse.bass as bass  # noqa: E402
import concourse.tile as tile  # noqa: E402
from concourse import bacc, mybir  # noqa: E402
from concourse import dve_ops as _dvo  # noqa: E402
from concourse.dve_spec import (  # noqa: E402
    Spec, Src0, Src1, C0, One, lower, sq, _has_src1,
)
from concourse.dve_uop import DveOpSpec  # noqa: E402
from concourse.bass_utils import run_bass_kernel_spmd  # noqa: E402

SQ_A, SQ_B = 5, 8
NEXP = 1 << 13

BF16 = mybir.dt.bfloat16
F32 = mybir.dt.float32
FP8 = mybir.dt.float8e4
NPBF16 = ml_dtypes.bfloat16
NPFP8 = ml_dtypes.float8_e4m3fn

B = 2            # batches
N = 2048         # tokens per batch (= 8*16*16, z-major)
C = 128          # channels
HEADS = 8
DH = 32          # head dim
NCORES = 8

# softmax routes by bias slab class kk = 2*ic - jj + 7 in [0, 13]
D_SET = frozenset({0, 1, 2, 3, 12, 13})     # DVE custom exp (16 tiles)
P_SET = frozenset()                         # ACT exp + Pool mul
I_SET = frozenset(range(14)) - D_SET - P_SET  # PE inject + ACT exp (48)
D_SLOT = {kk: s for s, kk in enumerate(sorted(D_SET))}
P_SLOT = {kk: s for s, kk in enumerate(sorted(P_SET))}
I_SLOT = {kk: s for s, kk in enumerate(sorted(I_SET))}


def _register_op(name, spec, subdim=False):
    for op in _dvo.OPS:
        if op.name == name:
            return op
    shas = {}
    for ver in ("v3", "v4"):
        uops = lower(spec, ver=ver)
        shas[ver] = DveOpSpec(name=name, opcode=1, uops=uops,
                              rd1_en=_has_src1(spec)).sha(ver)
    op = _dvo.DveOp(name=name, spec=spec, subdim=subdim, uops_sha=shas)
    _dvo.OPS.append(op)
    _dvo.CUSTOM_DVE_SPECS[name] = spec
    _dvo._SUB_OPCODE_FOR_NAME[name] = (
        _dvo._CUSTOM_DVE_ROW_BASE + len(_dvo.OPS) - 1)
    return op


def _ref_exp2n_a(in0, in1, s0, s1, imm2):
    t = ((in0.astype(np.float32) + in1.astype(np.float32)) * np.float32(s0)
         + np.float32(1.0)).astype(np.float32)
    for _ in range(SQ_A):
        t = (t * t).astype(np.float32)
    return t


def _ref_exp2n_b(in0, in1, s0, s1, imm2):
    t = in0.astype(np.float32)
    for _ in range(SQ_B):
        t = (t * t).astype(np.float32)
    return t


def _build_ops():
    body_a = (Src0 + Src1) * C0 + One
    for _ in range(SQ_A):
        body_a = sq(body_a)
    op_a = _register_op("EXP2N_A_ANT", Spec(body=body_a,
                                            reference=_ref_exp2n_a))
    body_b = Src0
    for _ in range(SQ_B):
        body_b = sq(body_b)
    op_b = _register_op("EXP2N_B_ANT", Spec(body=body_b,
                                            reference=_ref_exp2n_b))
    return op_a, op_b


EXP2N_A, EXP2N_B = _build_ops()


def _route(ic, jj):
    kk = 2 * ic - jj + 7
    if kk in D_SET:
        return "D", kk
    if kk in P_SET:
        return "P", kk
    return "I", kk


def _jj_order(ic):
    """Spread D-route tiles between ACT-route tiles so the DVE and ACT
    softmax pipelines run concurrently."""
    djj = [jj for jj in range(8) if _route(ic, jj)[0] == "D"]
    ajj = [jj for jj in range(8) if _route(ic, jj)[0] != "D"]
    if not djj:
        return ajj
    nd, na = len(djj), len(ajj)
    pos_d = {int(t * (8 / nd) + (8 / nd) / 2) for t in range(nd)}
    out = []
    di = ai = 0
    for k in range(8):
        if k in pos_d and di < nd:
            out.append(djj[di]); di += 1
        elif ai < na:
            out.append(ajj[ai]); ai += 1
        else:
            out.append(djj[di]); di += 1
    return out


# ---------------------------------------------------------------------------
# device graph
# ---------------------------------------------------------------------------


def _build():
    nc = bacc.Bacc(None, target_bir_lowering=False, debug=False)

    xt_e = nc.declare_dram_parameter("xt", [C, B * N], BF16, isOutput=False)
    w3_e = nc.declare_dram_parameter("w3", [C, 96], BF16, isOutput=False)
    waug_e = nc.declare_dram_parameter("waug", [DH + 1, C + 1], BF16,
                                       isOutput=False)
    ident_e = nc.declare_dram_parameter("ident", [128, 256], FP8,
                                        isOutput=False)
    logb_e = nc.declare_dram_parameter("logb", [128, len(D_SLOT) * 1024],
                                       BF16, isOutput=False)
    expb_e = (nc.declare_dram_parameter("expb", [128, len(P_SLOT) * 1024],
                                        BF16, isOutput=False)
              if P_SLOT else None)
    binj_e = nc.declare_dram_parameter("binj", [128, len(I_SLOT) * 2048],
                                       FP8, isOutput=False)
    out_e = nc.declare_dram_parameter("out", [B * N, C + 1], F32,
                                      isOutput=True)

    with tile.TileContext(nc) as tc:
        with tc.tile_pool(name="persist", bufs=1) as persist:
            w3 = persist.tile([C, 96], BF16)
            nc.sync.dma_start(w3[:], w3_e[:])
            # PE p-state warm-up on dummy matmuls during the startup DMA wait
            with tc.tile_pool(name="warm", bufs=1, space="PSUM") as warm:
                wsb = persist.tile([128, 256], BF16)
                nc.vector.memset(wsb[:], 0.0)
                wps = warm.tile([128, 256], F32)
                for _ in range(20):
                    nc.tensor.matmul(wps[:], lhsT=wsb[:, 0:128], rhs=wsb[:],
                                     start=True, stop=True)
            xt = persist.tile([C, B * N], BF16)
            for ch in range(4):
                nc.sync.dma_start(xt[:, ch * 1024:(ch + 1) * 1024],
                                  xt_e[:, ch * 1024:(ch + 1) * 1024])
            waug = persist.tile([DH + 1, C + 1], BF16)
            nc.sync.dma_start(waug[:], waug_e[:])
            ident = persist.tile([128, 256], FP8)
            nc.sync.dma_start(ident[:], ident_e[:])

            # bias slabs, in first-use order (b0 walks kk 7..0, then 9,8,
            # 11,10, 13,12)
            logb = persist.tile([128, len(D_SLOT) * 1024], BF16)
            expb = (persist.tile([128, len(P_SLOT) * 1024], BF16)
                    if P_SLOT else None)
            binj = persist.tile([128, len(I_SLOT) * 2048], FP8)
            for kk in (7, 6, 5, 4, 3, 2, 1, 0, 9, 8, 11, 10, 13, 12):
                if kk in D_SLOT:
                    s = D_SLOT[kk]
                    nc.sync.dma_start(logb[:, s * 1024:(s + 1) * 1024],
                                      logb_e[:, s * 1024:(s + 1) * 1024])
                if kk in P_SLOT:
                    s = P_SLOT[kk]
                    nc.sync.dma_start(expb[:, s * 1024:(s + 1) * 1024],
                                      expb_e[:, s * 1024:(s + 1) * 1024])
                if kk in I_SLOT:
                    s = I_SLOT[kk]
                    nc.sync.dma_start(binj[:, s * 2048:(s + 1) * 2048],
                                      binj_e[:, s * 2048:(s + 1) * 2048])

            # preload the Exp activation table during phase 1
            scratch = persist.tile([128, 1], F32)
            nc.vector.memset(scratch[:], 0.0)
            nc.scalar.activation(scratch[:], scratch[:],
                                 mybir.ActivationFunctionType.Exp)

            qkT = [persist.tile([64, N], BF16, tag=f"qkT{b}", name=f"qkT{b}")
                   for b in range(B)]
            kT = [persist.tile([32, N], BF16, tag=f"kT{b}", name=f"kT{b}")
                  for b in range(B)]
            vaug = [persist.tile([128, 16 * 33], BF16, tag=f"vaug{b}",
                                 name=f"vaug{b}") for b in range(B)]

            # ---- phase 1: qkv projections -------------------------------
            with tc.tile_pool(name="ph1", bufs=2, space="PSUM") as ph1, \
                 tc.tile_pool(name="ph1v", bufs=2, space="PSUM") as ph1v:
                for b in range(B):
                    nc.gpsimd.memset(vaug[b][:], 1.0)
                    for hh in range(2):
                        qk_ps = ph1.tile([64, 1024], F32, tag="qk_ps",
                                         name="qk_ps")
                        for ch in range(2):
                            col = hh * 1024 + ch * 512
                            nc.tensor.matmul(
                                qk_ps[:, ch * 512:(ch + 1) * 512],
                                lhsT=w3[:, 0:64],
                                rhs=xt[:, b * N + col:b * N + col + 512],
                                start=True, stop=True)
                        # evict halves on ACT and DVE in parallel
                        if hh == 0:
                            nc.scalar.activation(
                                qkT[b][:, 0:1024], qk_ps[:],
                                mybir.ActivationFunctionType.Copy)
                        else:
                            nc.vector.tensor_copy(qkT[b][:, 1024:2048],
                                                  qk_ps[:])
                        # k to base partition 0 (PE needs lhsT/rhs co-based)
                        nc.gpsimd.dma_start(
                            kT[b][:, hh * 1024:(hh + 1) * 1024],
                            qkT[b][32:64, hh * 1024:(hh + 1) * 1024])
                    for tt in range(4):
                        v_ps = ph1v.tile([128, 128], F32, tag="v_ps",
                                         name="v_ps")
                        for u in range(4):
                            nt = tt * 4 + u
                            nc.tensor.matmul(
                                v_ps[:, u * 32:(u + 1) * 32],
                                lhsT=xt[:, b * N + nt * 128:
                                        b * N + (nt + 1) * 128],
                                rhs=w3[:, 64:96],
                                start=(u == 0), stop=(u == 3),
                                skip_group_check=True)
                        dst = vaug[b][:, tt * 132:(tt + 1) * 132]
                        dst = dst.rearrange("p (f c) -> p f c", f=4)[:, :, 0:DH]
                        src = v_ps[:].rearrange("p (f c) -> p f c", f=4)
                        nc.vector.tensor_copy(dst, src)

            # ---- phase 2: attention ------------------------------------
            with (
                tc.tile_pool(name="score_a", bufs=2, space="PSUM") as score_a,
                tc.tile_pool(name="score_d", bufs=1, space="PSUM") as score_d,
                tc.tile_pool(name="outps", bufs=1, space="PSUM") as out_pool,
                tc.tile_pool(name="proj", bufs=1, space="PSUM") as proj_pool,
                tc.tile_pool(name="sb2", bufs=3) as sb2,
                tc.tile_pool(name="sb3", bufs=2) as sb3,
            ):
                def epilogue_early(out_ps):
                    # evict the PV accumulator and launch the xbar transposes;
                    # returns the transposed tile for epilogue_late.
                    osb = sb3.tile([128, 512], BF16, tag="osb", name="osb")
                    ov = osb[:].rearrange("p (f c) -> p f c", f=4)[:, :, 0:DH + 1]
                    nc.vector.tensor_copy(
                        ov, out_ps[:].rearrange("p (f c) -> p f c", f=4))
                    tT = sb3.tile([128, 512], BF16, tag="tT", name="tT")
                    for ib in range(4):
                        nc.sync.dma_start_transpose(
                            tT[:, ib * 128:(ib + 1) * 128],
                            osb[:, ib * 128:(ib + 1) * 128])
                    return tT

                def epilogue_late(b, ic, tT):
                    for half in range(2):
                        proj_ps = proj_pool.tile([128, 2 * (C + 1)], F32,
                                                 tag="proj", name="proj")
                        for u in range(2):
                            it = 2 * half + u
                            nc.tensor.matmul(
                                proj_ps[:, u * (C + 1):(u + 1) * (C + 1)],
                                lhsT=tT[0:DH + 1, it * 128:(it + 1) * 128],
                                rhs=waug[:], start=True, stop=True)
                        psb = sb3.tile([128, 2 * (C + 1)], F32, tag="psb",
                                       name="psb")
                        nc.vector.tensor_copy(psb[:], proj_ps[:])
                        row = b * N + ic * 512 + 2 * half * 128
                        dst = out_e[row:row + 256, :].rearrange(
                            "(u p) c -> p u c", u=2)
                        nc.sync.dma_start(
                            dst, psb[:].rearrange("p (u c) -> p u c", u=2))

                steps = []
                for b in range(B):
                    for ic in range(4):
                        order = _jj_order(ic)
                        for s, jj in enumerate(order):
                            steps.append((b, ic, jj, s))
                out_ps_of = {}
                carries = []          # deferred consumer+PV closures (2-deep)
                pending_early = None  # (b, ic) chunk awaiting evict+transpose
                pending_late = None   # (b, ic, tT) awaiting proj+store
                for (b, ic, jj, s) in steps:
                    if s == 0:
                        out_ps_of[(b, ic)] = out_pool.tile(
                            [128, 4 * (DH + 1)], F32, name="out_ps",
                            tag="out_ps")
                    route, kk = _route(ic, jj)
                    if route == "D":
                        score_ps = score_d.tile([128, 1024], F32,
                                                name="score_d", tag="score_d")
                    else:
                        score_ps = score_a.tile([128, 1024], F32,
                                                name="score_a", tag="score_a")
                    for g in range(2):
                        jt = 2 * jj + g
                        nc.tensor.matmul(
                            score_ps[:, g * 512:(g + 1) * 512],
                            lhsT=kT[b][:, jt * 128:(jt + 1) * 128],
                            rhs=qkT[b][0:32, ic * 512:(ic + 1) * 512],
                            start=True, stop=(route != "I"),
                            skip_group_check=True)
                        if route == "I":
                            sl = I_SLOT[kk]
                            rhs = binj[:, sl * 2048 + g * 1024:
                                       sl * 2048 + (g + 1) * 1024]
                            nc.tensor.matmul(
                                score_ps[:, g * 512:(g + 1) * 512],
                                lhsT=ident[:].rearrange("p (e m) -> p e m",
                                                        e=2),
                                rhs=rhs.rearrange("p (e n) -> p e n", e=2),
                                start=False, stop=True,
                                perf_mode=mybir.MatmulPerfMode.DoubleRow,
                                skip_group_check=True)
                    if len(carries) >= 2:
                        carries.pop(0)()
                    if s == 0 and carries:
                        # drain the previous chunk's tail so its epilogue can
                        # launch with maximal transpose headroom
                        carries.pop(0)()
                    if pending_early is not None and s >= 1:
                        pb, pic = pending_early
                        tT = epilogue_early(out_ps_of[(pb, pic)])
                        pending_late = (pb, pic, tT)
                        pending_early = None
                    if pending_late is not None and s >= 6:
                        epilogue_late(*pending_late)
                        pending_late = None

                    def emit_rest(b=b, ic=ic, jj=jj, s=s, route=route, kk=kk,
                                  score_ps=score_ps):
                        expT = sb2.tile([128, 1024], BF16, tag="expT",
                                        name="expT")
                        if route == "I":
                            nc.scalar.activation(
                                expT[:], score_ps[:],
                                mybir.ActivationFunctionType.Exp)
                        elif route == "P":
                            expS = sb2.tile([128, 1024], BF16, tag="expS",
                                            name="expS", bufs=2)
                            nc.scalar.activation(
                                expS[:], score_ps[:],
                                mybir.ActivationFunctionType.Exp)
                            sl = P_SLOT[kk]
                            nc.gpsimd.tensor_mul(
                                expT[:], expS[:],
                                expb[:, sl * 1024:(sl + 1) * 1024])
                        else:
                            t32 = sb2.tile([128, 1024], F32, tag="t32",
                                           name="t32", bufs=2)
                            sl = D_SLOT[kk]
                            nc.vector._custom_dve(
                                EXP2N_A, out=t32[:], in0=score_ps[:],
                                in1=logb[:, sl * 1024:(sl + 1) * 1024],
                                s0=1.0 / NEXP)
                            nc.vector._custom_dve(EXP2N_B, out=expT[:],
                                                  in0=t32[:])
                        out_ps = out_ps_of[(b, ic)]
                        for g in range(2):
                            jt = 2 * jj + g
                            for ib in range(4):
                                nc.tensor.matmul(
                                    out_ps[:, ib * (DH + 1):
                                           (ib + 1) * (DH + 1)],
                                    lhsT=expT[:, g * 512 + ib * 128:
                                              g * 512 + (ib + 1) * 128],
                                    rhs=vaug[b][:, jt * 33:(jt + 1) * 33],
                                    start=(s == 0 and g == 0 and ib == 0),
                                    stop=(s == 7 and g == 1 and ib == 3),
                                    skip_group_check=True)

                    carries.append(emit_rest)
                    if s == 7:
                        pending_early = (b, ic)
                for c in carries:
                    c()
                pb, pic = pending_early
                tT = epilogue_early(out_ps_of[(pb, pic)])
                if pending_late is not None:
                    epilogue_late(*pending_late)
                epilogue_late(pb, pic, tT)

    nc.compile()
    return nc


_NC = None


def _get_nc():
    global _NC
    if _NC is None:
        _NC = _build()
    return _NC


# ---------------------------------------------------------------------------
# host side
# ---------------------------------------------------------------------------

D3, H3, W3 = 8, 16, 16


def _bias_tile(table_h, kk):
    """Raw bias values for slab class kk: [128 p, 2 g, 512 ih] -> [128, 1024].

    scoresT tile: partition p = j within j-tile, col g*512+ih; j-token =
    jj*256 + g*128 + p (z-plane jj), i-token = ic*512 + ih.
    dz + 7 = kk + (ih >= 256).
    """
    p = np.arange(128)[:, None, None]
    g = np.arange(2)[None, :, None]
    ih = np.arange(512)[None, None, :]
    dz7 = kk + (ih // 256)
    pj = g * 128 + p
    yj, xj = pj // 16, pj % 16
    pi = ih % 256
    yi, xi = pi // 16, pi % 16
    idx = dz7 * ((2 * H3 - 1) * (2 * W3 - 1)) + (yi - yj + 15) * (2 * W3 - 1) \
        + (xi - xj + 15)
    return table_h[idx].reshape(128, 1024)


def _prep_in_maps(x, w_qkv, rel_table, w_out, b_out):
    x = np.asarray(x, np.float32)
    w_qkv = np.asarray(w_qkv, np.float32)
    rel_table = np.asarray(rel_table, np.float32)
    w_out = np.asarray(w_out, np.float32)

    scale = DH ** -0.5
    xt = np.ascontiguousarray(
        x.transpose(2, 0, 1).reshape(C, B * N)).astype(NPBF16)

    ident = np.zeros((128, 2, 128), np.float32)
    ident[:, 0, :] = np.eye(128)
    ident = np.ascontiguousarray(ident.reshape(128, 256)).astype(NPFP8)

    in_maps = []
    for hc in range(NCORES):
        w3 = np.concatenate([
            w_qkv[:, hc * DH:(hc + 1) * DH] * scale,
            w_qkv[:, 256 + hc * DH: 256 + (hc + 1) * DH],
            w_qkv[:, 512 + hc * DH: 512 + (hc + 1) * DH],
        ], axis=1).astype(NPBF16)
        waug = np.zeros((DH + 1, C + 1), np.float32)
        waug[0:DH, 0:C] = w_out[hc * DH:(hc + 1) * DH, :]
        waug[DH, C] = 1.0
        th = rel_table[:, hc]
        logb = np.zeros((128, len(D_SLOT) * 1024), np.float32)
        for kk, s in D_SLOT.items():
            logb[:, s * 1024:(s + 1) * 1024] = _bias_tile(th, kk)
        binj = np.zeros((128, len(I_SLOT), 2, 2, 512), np.float32)
        for kk, s in I_SLOT.items():
            binj[:, s, :, 0, :] = _bias_tile(th, kk).reshape(128, 2, 512)
        binj = np.ascontiguousarray(
            binj.reshape(128, len(I_SLOT) * 2048)).astype(NPFP8)
        im = {
            "xt": xt,
            "w3": np.ascontiguousarray(w3),
            "waug": waug.astype(NPBF16),
            "ident": ident,
            "logb": logb.astype(NPBF16),
            "binj": binj,
        }
        if P_SLOT:
            expb = np.zeros((128, len(P_SLOT) * 1024), np.float32)
            for kk, s in P_SLOT.items():
                expb[:, s * 1024:(s + 1) * 1024] = np.exp(_bias_tile(th, kk))
            im["expb"] = expb.astype(NPBF16)
        in_maps.append(im)
    return in_maps


def _run(in_maps, **kwargs):
    nc = _get_nc()
    return run_bass_kernel_spmd(nc, in_maps, core_ids=list(range(NCORES)),
                                **kwargs)


def _postprocess(res, b_out):
    acc = np.zeros((B * N, C), np.float64)
    for i in range(NCORES):
        o = res.results[i]["out"].astype(np.float64)
        acc += o[:, 0:C] / o[:, C:C + 1]
    acc += np.asarray(b_out, np.float64)[None, :]
    return acc.reshape(B, N, C).astype(np.float32)


def kernel(x, w_qkv, rel_table, w_out, b_out, d=None, h=None, w=None):
    in_maps = _prep_in_maps(x, w_qkv, rel_table, w_out, b_out)
    res = _run(in_maps)
    return _postprocess(res, b_out)
